# revision 1
# baseline (speedup 1.0000x reference)
"""Trainium2 Bass kernel for nn_ClassifierModel (nms_detection).

Computation (reference):
    h    = relu(features @ conv_w + conv_b)        # (B,H,W,C)@(C,D) -> (B,H,W,D)
    flat = h.reshape(B, F)                         # F = H*W*D = 401408
    cls  = flat @ cls_w + cls_b                    # (B, 64)
    bbox = flat @ bbox_w + bbox_b                  # (B, 128)
    <tiny postprocessing with roi -> (B, P, 5)>

Sharding: the flatten (contraction) dim F is split across the 8 cores by
slicing H into 8 chunks of 28 rows. Each core computes its conv slice and a
partial (B, 192) product against its slice of [cls_w | bbox_w]; the host sums
the 8 partials and runs the tiny postprocessing. This reads each dense-weight
element exactly once across the machine (the weights dominate HBM traffic).

Per-core device layout (matmul operands bf16, accumulation fp32 in PSUM):
    featT  (4,128,NB) : features slice, transposed to (c, pix*B+b) columns, bf16
    convw  (4,128,256): conv_w k-tiles (c on partitions), bf16
    convb  (2,128,1)  : conv_b halves (d on partitions), fp32
    wmat   (128,NT*192): [cls|bbox] rows f-tiled in q-major consumption order
    out    (16,192)   : partial fp32 [cls|bbox] sums for this core's f range

Stage 1 produces h^T with d on partitions and (pix, b) on the free axis; a
128-partition f-tile of flat^T is then exactly hT[q][:, pix*16:(pix+1)*16],
so stage 2 needs no transposes at all. Stage 2 consumes f-tiles in q-major
order (all q=0 tiles, then q=1) so it can start as soon as hT[0] exists; the
host lays wmat out in the same order. The W stream is chunked with small tail
chunks so the final chunk's matmul tail is short.
"""

import numpy as np

B = 16
H, W, C = 224, 7, 512
D = 256
P = 32
NCORES = 8
HSH = H // NCORES          # 28 rows of H per core
PIX = HSH * W              # 196 pixels per core per batch
FLOC = PIX * D             # 50176 contraction elements per core
NB = PIX * B               # 3136 stage-1 moving columns
NT = FLOC // 128           # 392 f-tiles per core
NQ = NT // 2               # 196 f-tiles per d-half
NTILE = 448                # stage-1 moving tile (3136 = 7*448)
CHUNKS = [42] * 8 + [28, 14, 7, 4, 3]   # W-stream chunks (sum = 392)
STRIDE = 16.0

_STATE = {}


def _build_module(reps=1):
    import concourse.mybir as mybir
    import concourse.tile as tile
    from concourse import bacc

    f32 = mybir.dt.float32
    bf16 = mybir.dt.bfloat16
    nc = bacc.Bacc("TRN2", target_bir_lowering=False, debug=False)

    featT = nc.dram_tensor("featT", [4, 128, NB], bf16, kind="ExternalInput")
    convw = nc.dram_tensor("convw", [4, 128, D], bf16, kind="ExternalInput")
    convb = nc.dram_tensor("convb", [2, 128, 1], f32, kind="ExternalInput")
    wmat = nc.dram_tensor("wmat", [128, NT * 192], bf16, kind="ExternalInput")
    if reps == 1:
        out = nc.dram_tensor("out", [16, 192], f32, kind="ExternalOutput")
    else:
        out = nc.dram_tensor("out", [reps, 16, 192], f32, kind="ExternalOutput")

    NTI = NB // NTILE  # 7 stage-1 n-tiles

    with tile.TileContext(nc) as tc:
        with (
            tc.tile_pool(name="res", bufs=2 if reps > 1 else 1) as res,
            tc.tile_pool(name="win", bufs=5) as win,
            tc.tile_pool(name="ps1", bufs=7, space="PSUM") as ps1p,
            tc.tile_pool(name="ps2", bufs=1, space="PSUM") as ps2p,
        ):
            for rep in range(reps):
                xts = []
                for t in range(4):
                    xt = res.tile([128, NB], bf16, tag=f"xt{t}", name=f"xt{t}")
                    nc.sync.dma_start(xt[:], featT[t])
                    xts.append(xt)
                cws = []
                for t in range(4):
                    cw = res.tile([128, D], bf16, tag=f"cw{t}", name=f"cw{t}")
                    nc.sync.dma_start(cw[:], convw[t])
                    cws.append(cw)
                cbs = []
                for q in range(2):
                    cb = res.tile([128, 1], f32, tag=f"cb{q}", name=f"cb{q}")
                    nc.sync.dma_start(cb[:], convb[q])
                    cbs.append(cb)
                hts = [res.tile([128, NB], bf16, tag=f"ht{q}", name=f"ht{q}")
                       for q in range(2)]

                # Stage 1, k-outer so matmuls start as soon as xt[0] lands:
                # hT[q][:, n-tile] = relu(conv_w[:, q-half].T @ featT + b)
                for q in range(2):
                    pss = [ps1p.tile([128, NTILE], f32, tag="ps",
                                     name=f"ps{q}_{n}") for n in range(NTI)]
                    for k in range(4):
                        for n in range(NTI):
                            nc.tensor.matmul(
                                pss[n][:],
                                cws[k][:, q * 128:(q + 1) * 128],
                                xts[k][:, n * NTILE:(n + 1) * NTILE],
                                start=(k == 0),
                                stop=(k == 3),
                            )
                    for n in range(NTI):
                        nc.scalar.activation(
                            hts[q][:, n * NTILE:(n + 1) * NTILE],
                            pss[n][:],
                            mybir.ActivationFunctionType.Relu,
                            bias=cbs[q],
                        )

                # Stage 2: acc(16,192) += hT-slice(128,16).T @ W-tile(128,192)
                # q-major f-tile order; W stream chunked per CHUNKS.
                acc = ps2p.tile([16, 192], f32, tag="acc", name="acc")
                pos = 0
                for ch in CHUNKS:
                    wc = win.tile([128, ch * 192], bf16, tag="wc", name="wc")
                    nc.sync.dma_start(
                        wc[:], wmat[:, pos * 192:(pos + ch) * 192])
                    for t in range(ch):
                        p_ = pos + t
                        q, pix = (0, p_) if p_ < NQ else (1, p_ - NQ)
                        nc.tensor.matmul(
                            acc[:],
                            hts[q][:, pix * 16:(pix + 1) * 16],
                            wc[:, t * 192:(t + 1) * 192],
                            start=(p_ == 0),
                            stop=(p_ == NT - 1),
                        )
                    pos += ch

                ot = res.tile([16, 192], f32, tag="ot", name="ot")
                nc.vector.tensor_copy(ot[:], acc[:])
                nc.sync.dma_start(out[:] if reps == 1 else out[rep], ot[:])

    nc.compile()
    return nc


def _prep_inputs(features, conv_w, conv_b, cls_w, bbox_w):
    import ml_dtypes

    f32 = np.float32
    bf16 = ml_dtypes.bfloat16
    features = np.asarray(features, dtype=f32).astype(bf16)
    conv_w = np.asarray(conv_w, dtype=f32).astype(bf16)
    conv_b = np.ascontiguousarray(conv_b, dtype=f32)

    convw_t = np.ascontiguousarray(conv_w.reshape(4, 128, D))
    convb_t = conv_b.reshape(2, 128, 1)

    in_maps = []
    for i in range(NCORES):
        fi = features[:, i * HSH:(i + 1) * HSH, :, :].reshape(B, PIX, C)
        featT = np.ascontiguousarray(fi.transpose(2, 1, 0).reshape(C, NB))

        # wmat block t holds W rows for the t-th f-tile in q-major order:
        # t < NQ -> f-tile 2t (q=0), else f-tile 2(t-NQ)+1 (q=1).
        wl = np.empty((128, NT, 192), dtype=bf16)
        r0, r1 = i * FLOC, (i + 1) * FLOC
        cw3 = cls_w[r0:r1].astype(bf16).reshape(NT, 128, 64)
        bw3 = bbox_w[r0:r1].astype(bf16).reshape(NT, 128, 128)
        wl[:, :NQ, :64] = cw3[0::2].transpose(1, 0, 2)
        wl[:, NQ:, :64] = cw3[1::2].transpose(1, 0, 2)
        wl[:, :NQ, 64:] = bw3[0::2].transpose(1, 0, 2)
        wl[:, NQ:, 64:] = bw3[1::2].transpose(1, 0, 2)

        in_maps.append({
            "featT": featT.reshape(4, 128, NB),
            "convw": convw_t,
            "convb": convb_t,
            "wmat": wl.reshape(128, NT * 192),
        })
    return in_maps


def _run_device(in_maps, trace=False, **kw):
    from concourse.bass_utils import run_bass_kernel_spmd

    if "nc" not in _STATE:
        _STATE["nc"] = _build_module()
    nc = _STATE["nc"]
    return run_bass_kernel_spmd(
        nc, in_maps, core_ids=list(range(NCORES)), trace=trace, **kw
    )


def _postprocess(partial, roi, cls_b, bbox_b):
    f32 = np.float32
    cls = partial[:, :64].astype(f32) + cls_b.astype(f32)
    bbox = partial[:, 64:].astype(f32) + bbox_b.astype(f32)

    obj = 1.0 / (1.0 + np.exp(-(cls[:, P:] - cls[:, :P]), dtype=f32))
    bb = bbox.reshape(B, 4, P).transpose(0, 2, 1)
    roi_img = roi.astype(f32) * f32(STRIDE)
    x = roi_img[:, :, 0] - bb[:, :, 1] * roi_img[:, :, 3]
    y = roi_img[:, :, 1]
    w = roi_img[:, :, 2] * np.exp(np.clip(bb[:, :, 2], -10.0, 10.0), dtype=f32)
    hh = roi_img[:, :, 3] * np.exp(np.clip(bb[:, :, 3], -10.0, 10.0), dtype=f32)
    return np.stack([x, y, w, hh, obj], axis=-1).astype(f32)


def kernel(features, roi, conv_w, conv_b, cls_w, cls_b, bbox_w, bbox_b):
    in_maps = _prep_inputs(features, conv_w, conv_b, cls_w, bbox_w)
    res = _run_device(in_maps)
    partial = np.zeros((B, 192), dtype=np.float64)
    for r in res.results:
        partial += np.asarray(r["out"], dtype=np.float64)
    return _postprocess(partial.astype(np.float32), np.asarray(roi),
                        np.asarray(cls_b), np.asarray(bbox_b))



# revision 2
# speedup vs baseline: 2.2796x; 2.2796x over previous
"""Trainium2 Bass kernel for nn_ClassifierModel (nms_detection).

Reference computation:
    h    = relu(features @ conv_w + conv_b)        # (B,H,W,C)@(C,D) -> (B,H,W,D)
    flat = h.reshape(B, F)                         # F = H*W*D = 401408
    cls  = flat @ cls_w + cls_b                    # (B, 64)
    bbox = flat @ bbox_w + bbox_b                  # (B, 128)
    <tiny postprocessing with roi -> (B, P, 5)>

Weight folding (host, exact f32 algebra a compiler could do statically):
  - objectness only needs sigmoid(cls1-cls0), so ship wdiff = cls_w[:,P:] -
    cls_w[:,:P] (32 cols) instead of 64 cls columns.
  - bb[:,:,0] is dead in the reference output (the first x assignment is
    overwritten), so bbox_w[:, :P] is never loaded.
  Device weight matrix = [wdiff | bbox_w[:,P:]]: 128 cols per f-tile.

Precision: all matmul operands are fp8 e4m3 (power-of-2 scaled so values sit
in the normal range; scales undone exactly on the host). PSUM accumulates
f32. Measured end-to-end rel err vs the f32 reference: ~9.6e-3.

Sharding: the flatten (contraction) dim F is split across the 8 cores by
slicing H into 8 chunks of 28 rows (196 pixels/core). Each core computes its
conv slice and a partial (16, 128) product against its slice of the folded
weights; the host sums the 8 partials and runs the tiny postprocessing. Each
dense-weight byte is read exactly once machine-wide, in fp8.

Per-core layout (fp8 = float8e4):
    xt4  [128,4,3136]: features slice, xt4[p,t,pix*16+b] = feat[c=t*128+p]
    cw4  [128,4,256] : 64*conv_w, cw4[p,t,d]
    cb2  [128,2] f32 : 64*conv_b halves
    wmat [128,392,128]: folded weights * 2^kw, middle index = pix*2 + i
                        (i = d half), cols = [wdiff(32) | bbox1,2,3(96)]
    out  [16,128] f32: partial [cls_diff | bbox123] * 2^(6+kw)

Stage 1 uses fp8 DoubleRow matmuls (two 128-deep c-tiles per matmul) to
produce hQ[p, pix*2+q, b] = fp8(64*h[d=q*128+p, pix, b]) via a strided
activation write. Stage 2 then contracts one full pixel (256 d-values) per
DoubleRow matmul: acc[16,128] += hQ[:,2pix:2pix+2,:].T @ wc[:,2t:2t+2,:].
The W stream is chunked (28 pixels per chunk) and double-buffered so the DMA
engines stay saturated; the kernel is HBM-bound at ~8.2 MB/core.
"""

import numpy as np

B = 16
H, W, C = 224, 7, 512
D = 256
P = 32
NCORES = 8
HSH = H // NCORES          # 28 rows of H per core
PIX = HSH * W              # 196 pixels per core per batch
FLOC = PIX * D             # 50176 contraction elements per core
NB = PIX * B               # 3136 stage-1 moving columns
NTILE = 448                # stage-1 moving tile (3136 = 7*448)
NTI = NB // NTILE          # 7 stage-1 n-tiles
CPIX = PIX // NTI          # 28 pixels per n-tile / W chunk
WCOL = 128                 # device weight columns: 32 clsdiff + 96 bbox
STRIDE = 16.0
HSCALE_LOG2 = 6            # h is carried as 64*h in fp8

_STATE = {}


def _build_module(reps=1):
    import concourse.mybir as mybir
    import concourse.tile as tile
    from concourse import bacc

    f32 = mybir.dt.float32
    fp8 = mybir.dt.float8e4
    DR = mybir.MatmulPerfMode.DoubleRow
    nc = bacc.Bacc("TRN2", target_bir_lowering=False, debug=False)

    xt4_d = nc.dram_tensor("xt4", [128, 4, NB], fp8, kind="ExternalInput")
    cw4_d = nc.dram_tensor("cw4", [128, 4, D], fp8, kind="ExternalInput")
    cb2_d = nc.dram_tensor("cb2", [128, 2], f32, kind="ExternalInput")
    wmat = nc.dram_tensor("wmat", [128, 2 * PIX, WCOL], fp8,
                          kind="ExternalInput")
    if reps == 1:
        out = nc.dram_tensor("out", [16, WCOL], f32, kind="ExternalOutput")
    else:
        out = nc.dram_tensor("out", [reps, 16, WCOL], f32,
                             kind="ExternalOutput")

    with tile.TileContext(nc) as tc:
        with (
            tc.tile_pool(name="res", bufs=2 if reps > 1 else 1) as res,
            tc.tile_pool(name="win", bufs=NTI) as win,
            tc.tile_pool(name="ps1", bufs=7, space="PSUM") as ps1p,
            tc.tile_pool(name="ps2", bufs=1, space="PSUM") as ps2p,
        ):
            for rep in range(reps):
                cw4 = res.tile([128, 4, D], fp8, tag="cw4", name="cw4")
                nc.sync.dma_start(cw4[:], cw4_d[:])
                cb2 = res.tile([128, 2], f32, tag="cb2", name="cb2")
                nc.sync.dma_start(cb2[:], cb2_d[:])
                xt4 = res.tile([128, 4, NB], fp8, tag="xt4", name="xt4")
                nc.sync.dma_start(xt4[:], xt4_d[:])
                wcs = []
                for c in range(NTI):
                    wc = win.tile([128, 2 * CPIX, WCOL], fp8, tag="wc",
                                  name=f"wc{c}")
                    nc.sync.dma_start(
                        wc[:], wmat[:, c * 2 * CPIX:(c + 1) * 2 * CPIX, :])
                    wcs.append(wc)

                hQ = res.tile([128, 2 * PIX, 16], fp8, tag="hQ", name="hQ")

                # Stage 1: hQ[:, 2*pix+q, b] = relu(64*(conv)+64*b) in fp8.
                # One DoubleRow matmul contracts two 128-deep c-tiles.
                for n in range(NTI):
                    for q in range(2):
                        ps = ps1p.tile([128, CPIX, 16], f32, tag="ps",
                                       name=f"ps{n}_{q}")
                        for j in range(2):
                            nc.tensor.matmul(
                                ps[:],
                                cw4[:, 2 * j:2 * j + 2,
                                    q * 128:(q + 1) * 128],
                                xt4[:, 2 * j:2 * j + 2,
                                    n * NTILE:(n + 1) * NTILE],
                                start=(j == 0),
                                stop=(j == 1),
                                perf_mode=DR,
                            )
                        nc.scalar.activation(
                            hQ[:, 2 * n * CPIX + q:2 * (n + 1) * CPIX:2, :],
                            ps[:],
                            mybir.ActivationFunctionType.Relu,
                            bias=cb2[:, q:q + 1],
                        )

                # Stage 2: one DoubleRow matmul per pixel contracts all 256
                # d-values: acc += hQ-pair.T @ W-pair.
                acc = ps2p.tile([16, WCOL], f32, tag="acc", name="acc")
                for c in range(NTI):
                    for t in range(CPIX):
                        pix = c * CPIX + t
                        nc.tensor.matmul(
                            acc[:],
                            hQ[:, 2 * pix:2 * pix + 2, :],
                            wcs[c][:, 2 * t:2 * t + 2, :],
                            start=(pix == 0),
                            stop=(pix == PIX - 1),
                            perf_mode=DR,
                        )

                ot = res.tile([16, WCOL], f32, tag="ot", name="ot")
                nc.vector.tensor_copy(ot[:], acc[:])
                nc.sync.dma_start(out[:] if reps == 1 else out[rep], ot[:])

    nc.compile()
    return nc


def _prep_inputs(features, conv_w, conv_b, cls_w, bbox_w):
    import ml_dtypes

    f32 = np.float32
    fp8 = ml_dtypes.float8_e4m3

    conv_w = np.asarray(conv_w, dtype=f32)
    conv_b = np.asarray(conv_b, dtype=f32)
    cls_w = np.asarray(cls_w, dtype=f32)
    bbox_w = np.asarray(bbox_w, dtype=f32)
    features = np.asarray(features, dtype=f32)

    # Folded, power-of-2-scaled weights (scale undone exactly on the host).
    wfull = np.concatenate([cls_w[:, P:] - cls_w[:, :P], bbox_w[:, P:]],
                           axis=1)  # (F, 128)
    kw = int(np.floor(np.log2(224.0 / np.abs(wfull).max())))
    wq = (wfull * f32(2.0 ** kw)).astype(fp8)

    cw4 = (conv_w * f32(64.0)).astype(fp8).reshape(4, 128, D)
    cw4 = np.ascontiguousarray(cw4.transpose(1, 0, 2))          # (128,4,D)
    cb2 = np.ascontiguousarray((conv_b * f32(64.0)).reshape(2, 128).T)

    in_maps = []
    for i in range(NCORES):
        fi = features[:, i * HSH:(i + 1) * HSH, :, :].astype(fp8)
        fi = fi.reshape(B, PIX, C).transpose(2, 1, 0).reshape(4, 128, NB)
        xt4 = np.ascontiguousarray(fi.transpose(1, 0, 2))       # (128,4,NB)

        wl = wq[i * FLOC:(i + 1) * FLOC].reshape(PIX, 2, 128, WCOL)
        wl = np.ascontiguousarray(wl.transpose(2, 0, 1, 3))     # (128,PIX,2,W)

        in_maps.append({
            "xt4": xt4,
            "cw4": cw4,
            "cb2": cb2,
            "wmat": wl.reshape(128, 2 * PIX, WCOL),
        })
    return in_maps, kw


def _run_device(in_maps, trace=False, **kw):
    from concourse.bass_utils import run_bass_kernel_spmd

    if "nc" not in _STATE:
        _STATE["nc"] = _build_module()
    nc = _STATE["nc"]
    return run_bass_kernel_spmd(
        nc, in_maps, core_ids=list(range(NCORES)), trace=trace, **kw
    )


def _postprocess(partial, roi, cls_b, bbox_b):
    f32 = np.float32
    cls_b = np.asarray(cls_b, dtype=f32)
    bbox_b = np.asarray(bbox_b, dtype=f32)

    cls_diff = partial[:, :P] + (cls_b[P:] - cls_b[:P])
    obj = 1.0 / (1.0 + np.exp(-cls_diff, dtype=f32))
    bb = partial[:, P:].reshape(B, 3, P) + bbox_b[P:].reshape(3, P)
    roi_img = np.asarray(roi, dtype=f32) * f32(STRIDE)
    x = roi_img[:, :, 0] - bb[:, 0, :] * roi_img[:, :, 3]
    y = roi_img[:, :, 1]
    w = roi_img[:, :, 2] * np.exp(np.clip(bb[:, 1, :], -10.0, 10.0), dtype=f32)
    hh = roi_img[:, :, 3] * np.exp(np.clip(bb[:, 2, :], -10.0, 10.0),
                                   dtype=f32)
    return np.stack([x, y, w, hh, obj], axis=-1).astype(f32)


def kernel(features, roi, conv_w, conv_b, cls_w, cls_b, bbox_w, bbox_b):
    in_maps, kw = _prep_inputs(features, conv_w, conv_b, cls_w, bbox_w)
    res = _run_device(in_maps)
    partial = np.zeros((B, WCOL), dtype=np.float64)
    for r in res.results:
        partial += np.asarray(r["out"], dtype=np.float64)
    partial *= 2.0 ** (-HSCALE_LOG2 - kw)
    return _postprocess(partial.astype(np.float32), roi, cls_b, bbox_b)


# revision 11
# speedup vs baseline: 2.4114x; 1.0578x over previous
"""Trainium2 Bass kernel for nn_ClassifierModel (nms_detection).

Reference computation:
    h    = relu(features @ conv_w + conv_b)        # (B,H,W,C)@(C,D) -> (B,H,W,D)
    flat = h.reshape(B, F)                         # F = H*W*D = 401408
    cls  = flat @ cls_w + cls_b                    # (B, 64)
    bbox = flat @ bbox_w + bbox_b                  # (B, 128)
    <tiny postprocessing with roi -> (B, P, 5)>

Weight folding (host, exact f32 algebra a compiler could do statically):
  - objectness only needs sigmoid(cls1-cls0), so ship wdiff = cls_w[:,P:] -
    cls_w[:,:P] (32 cols) instead of 64 cls columns.
  - bb[:,:,0] is dead in the reference output (the first x assignment is
    overwritten), so bbox_w[:, :P] is never loaded.
  Device weight matrix = [wdiff | bbox_w[:,P:]]: 128 cols per f-tile.

Precision: all matmul operands are fp8 e4m3 (power-of-2 scaled so values sit
in the normal range; scales undone exactly on the host). PSUM accumulates
f32. Measured end-to-end rel err vs the f32 reference: ~9.6e-3 (gate 2e-2).

Sharding: the flatten (contraction) dim F is split across the 8 cores by
slicing H into 8 chunks of 28 rows (196 pixels/core). Each core computes its
conv slice and a partial (16, 128) product against its slice of the folded
weights; the host sums the 8 partials and runs the tiny postprocessing. Each
dense-weight byte is read exactly once machine-wide, in fp8: ~8.2 MB/core,
which at the 360 GB/s DMA roofline is ~22.7 us — the kernel is HBM-bound.

Per-core layout (fp8 = float8e4):
    xt4  [128,4,3136]: features slice, xt4[p,t,pix*16+b] = feat[c=t*128+p]
    cw4  [128,4,256] : 64*conv_w, cw4[p,t,d]
    cb2  [128,2] f32 : 64*conv_b halves
    wmat [128,392,128]: folded weights * 2^kw, middle index = pix*2 + i
                        (i = d half), cols = [wdiff(32) | bbox1,2,3(96)]
    out  [16,128] f32: partial [cls_diff | bbox123] * 2^(6+kw)

Stage 1 uses fp8 DoubleRow matmuls (two 128-deep c-tiles per matmul) to
produce hQ[p, pix*2+q, b] = fp8(64*h[d=q*128+p, pix, b]) via a strided
activation write. Stage 2 contracts one full pixel (256 d-values) per
DoubleRow matmul: acc[16,128] += hQ[:,2pix:2pix+2,:].T @ wc[:,2t:2t+2,:].

Schedule details (trace-driven):
  - DMA issue order: xt4 first (its long transfer hides the 625+650 ns
    HWDGE/DGE pipeline of the small cw4/cb2 DMAs), then the W stream.
  - The W stream ends with several tiny 2-pixel chunks so only ~54 ns of
    matmul work remains after the last byte (+900 ns DMA-sem latency).
  - The final [16,128] store is a pre-prepared SWDGE scatter-add fired with
    trigger_dma, skipping the ~1.3 us HWDGE+DGE issue latency a plain
    dma_start would add after the last accumulation. DRAM out is zeroed by
    a small DMA early in the stream (scatter ADDs into it).
"""

import numpy as np

B = 16
H, W, C = 224, 7, 512
D = 256
P = 32
NCORES = 8
HSH = H // NCORES          # 28 rows of H per core
PIX = HSH * W              # 196 pixels per core per batch
FLOC = PIX * D             # 50176 contraction elements per core
NB = PIX * B               # 3136 stage-1 moving columns
NTILE = 448                # stage-1 moving tile (3136 = 7*448)
NTI = NB // NTILE          # 7 stage-1 n-tiles
WCOL = 128                 # device weight columns: 32 clsdiff + 96 bbox
# W-stream chunks in pixels (sum = 196). The tapered tail keeps post-stream
# matmul work tiny while keeping the total DMA count low: HWDGE DMAs
# round-robin over 8 DMAHW lanes and each waits for the DMA 8 positions
# earlier to fully complete (+900 ns sem), so many small DMAs serialize.
CHUNKS = [28] * 6 + [12, 8, 4, 2, 2]
STRIDE = 16.0
HSCALE_LOG2 = 6            # h is carried as 64*h in fp8
USE_TRIGGER = False        # SWDGE scatter-add + trigger_dma for the output:
                           # structurally deadlocks under TileContext (the
                           # prep ticks a DMASW proc lane no instruction
                           # increments, and the SP exit drain waits on it),
                           # so the plain dma_start tail is used instead.

_STATE = {}


def _build_module(reps=1):
    import concourse.mybir as mybir
    import concourse.tile as tile
    from concourse import bacc

    f32 = mybir.dt.float32
    i16 = mybir.dt.int16
    fp8 = mybir.dt.float8e4
    DR = mybir.MatmulPerfMode.DoubleRow
    nc = bacc.Bacc("TRN2", target_bir_lowering=False, debug=False)

    xt4_d = nc.dram_tensor("xt4", [128, 4, NB], fp8, kind="ExternalInput")
    cw4_d = nc.dram_tensor("cw4", [128, 4, D], fp8, kind="ExternalInput")
    cb2_d = nc.dram_tensor("cb2", [128, 2], f32, kind="ExternalInput")
    wmat = nc.dram_tensor("wmat", [128, 2 * PIX, WCOL], fp8,
                          kind="ExternalInput")
    if reps == 1:
        out = nc.dram_tensor("out", [16, WCOL], f32, kind="ExternalOutput")
    else:
        out = nc.dram_tensor("out", [reps, 16, WCOL], f32,
                             kind="ExternalOutput")

    with tile.TileContext(nc) as tc:
        with (
            tc.tile_pool(name="res", bufs=2 if reps > 1 else 1) as res,
            tc.tile_pool(name="winb", bufs=6) as winb,
            tc.tile_pool(name="wins", bufs=5) as wins,
            tc.tile_pool(name="ps1", bufs=7, space="PSUM") as ps1p,
            tc.tile_pool(name="ps2", bufs=1, space="PSUM") as ps2p,
        ):
            for rep in range(reps):
                out_r = out[:] if reps == 1 else out[rep]

                # Output staging: ot spans 128 partitions because the SWDGE
                # scatter-add reads its full 128-partition footprint.
                ot = res.tile([128, 1, WCOL], f32, tag="ot", name="ot")
                dma_sem = None
                if USE_TRIGGER:
                    idxs = res.tile([128, 1], i16, tag="idxs", name="idxs")
                    nc.gpsimd.iota(idxs[:], pattern=[[1, 1]], base=0,
                                   channel_multiplier=1)
                    nc.gpsimd.memset(ot[:], 0.0)

                # xt4 first: its long transfer hides the DGE pipelining of
                # the small DMAs behind it.
                xt4 = res.tile([128, 4, NB], fp8, tag="xt4", name="xt4")
                nc.sync.dma_start(xt4[:], xt4_d[:])
                cw4 = res.tile([128, 4, D], fp8, tag="cw4", name="cw4")
                nc.sync.dma_start(cw4[:], cw4_d[:])
                cb2 = res.tile([128, 2], f32, tag="cb2", name="cb2")
                nc.sync.dma_start(cb2[:], cb2_d[:])
                if USE_TRIGGER:
                    # Zero DRAM out early; the final scatter ADDs into it.
                    # Emitted BEFORE the prep so the write-write ordering is
                    # carried by the trigger (the prep defers its data deps),
                    # not by an unsatisfiable wait on the prep's DMASW lane.
                    nc.sync.dma_start(out_r, ot[0:16, 0, :])
                    dma_sem = nc.alloc_semaphore(f"out_dma_{rep}")
                    nc.gpsimd.dma_scatter_add(
                        out_r, ot[:], idxs[:], 16, 16, WCOL,
                        prepare_only=True, sem=dma_sem,
                    )
                wcs = []
                pos = 0
                for c, npix in enumerate(CHUNKS):
                    pool = winb if npix > 12 else wins
                    wc = pool.tile([128, 2 * npix, WCOL], fp8,
                                   tag=f"wc{npix}", name=f"wc{c}")
                    nc.sync.dma_start(
                        wc[:], wmat[:, 2 * pos:2 * (pos + npix), :])
                    wcs.append(wc)
                    pos += npix

                hQ = res.tile([128, 2 * PIX, 16], fp8, tag="hQ", name="hQ")

                # Stage 1: hQ[:, 2*pix+q, b] = relu(64*conv + 64*b) in fp8.
                # One DoubleRow matmul contracts two 128-deep c-tiles.
                for n in range(NTI):
                    for q in range(2):
                        ps = ps1p.tile([128, NTILE // 16, 16], f32, tag="ps",
                                       name=f"ps{n}_{q}")
                        for j in range(2):
                            nc.tensor.matmul(
                                ps[:],
                                cw4[:, 2 * j:2 * j + 2,
                                    q * 128:(q + 1) * 128],
                                xt4[:, 2 * j:2 * j + 2,
                                    n * NTILE:(n + 1) * NTILE],
                                start=(j == 0),
                                stop=(j == 1),
                                perf_mode=DR,
                            )
                        nc.scalar.activation(
                            hQ[:, 2 * n * (NTILE // 16) + q:
                               2 * (n + 1) * (NTILE // 16):2, :],
                            ps[:],
                            mybir.ActivationFunctionType.Relu,
                            bias=cb2[:, q:q + 1],
                        )

                # Stage 2: one DoubleRow matmul per pixel contracts all 256
                # d-values: acc += hQ-pair.T @ W-pair.
                acc = ps2p.tile([16, WCOL], f32, tag="acc", name="acc")
                pix = 0
                for c, npix in enumerate(CHUNKS):
                    for t in range(npix):
                        nc.tensor.matmul(
                            acc[:],
                            hQ[:, 2 * pix:2 * pix + 2, :],
                            wcs[c][:, 2 * t:2 * t + 2, :],
                            start=(pix == 0),
                            stop=(pix == PIX - 1),
                            perf_mode=DR,
                        )
                        pix += 1

                nc.vector.tensor_copy(ot[0:16, 0, :], acc[:])
                if USE_TRIGGER:
                    nc.gpsimd.trigger_dma(count=None)
                    nc.gpsimd.wait_ge(dma_sem, 16 * (rep + 1))
                else:
                    nc.sync.dma_start(out_r, ot[0:16, 0, :])

    nc.compile()
    return nc


def _prep_inputs(features, conv_w, conv_b, cls_w, bbox_w):
    import ml_dtypes

    f32 = np.float32
    fp8 = ml_dtypes.float8_e4m3

    conv_w = np.asarray(conv_w, dtype=f32)
    conv_b = np.asarray(conv_b, dtype=f32)
    cls_w = np.asarray(cls_w, dtype=f32)
    bbox_w = np.asarray(bbox_w, dtype=f32)
    features = np.asarray(features, dtype=f32)

    # Folded, power-of-2-scaled weights (scale undone exactly on the host).
    wfull = np.concatenate([cls_w[:, P:] - cls_w[:, :P], bbox_w[:, P:]],
                           axis=1)  # (F, 128)
    kw = int(np.floor(np.log2(224.0 / np.abs(wfull).max())))
    wq = (wfull * f32(2.0 ** kw)).astype(fp8)

    cw4 = (conv_w * f32(64.0)).astype(fp8).reshape(4, 128, D)
    cw4 = np.ascontiguousarray(cw4.transpose(1, 0, 2))          # (128,4,D)
    cb2 = np.ascontiguousarray((conv_b * f32(64.0)).reshape(2, 128).T)

    in_maps = []
    for i in range(NCORES):
        fi = features[:, i * HSH:(i + 1) * HSH, :, :].astype(fp8)
        fi = fi.reshape(B, PIX, C).transpose(2, 1, 0).reshape(4, 128, NB)
        xt4 = np.ascontiguousarray(fi.transpose(1, 0, 2))       # (128,4,NB)

        wl = wq[i * FLOC:(i + 1) * FLOC].reshape(PIX, 2, 128, WCOL)
        wl = np.ascontiguousarray(wl.transpose(2, 0, 1, 3))     # (128,PIX,2,W)

        in_maps.append({
            "xt4": xt4,
            "cw4": cw4,
            "cb2": cb2,
            "wmat": wl.reshape(128, 2 * PIX, WCOL),
        })
    return in_maps, kw


def _run_device(in_maps, trace=False, **kw):
    from concourse.bass_utils import run_bass_kernel_spmd

    if "nc" not in _STATE:
        _STATE["nc"] = _build_module()
    nc = _STATE["nc"]
    return run_bass_kernel_spmd(
        nc, in_maps, core_ids=list(range(NCORES)), trace=trace, **kw
    )


def _postprocess(partial, roi, cls_b, bbox_b):
    f32 = np.float32
    cls_b = np.asarray(cls_b, dtype=f32)
    bbox_b = np.asarray(bbox_b, dtype=f32)

    cls_diff = partial[:, :P] + (cls_b[P:] - cls_b[:P])
    obj = 1.0 / (1.0 + np.exp(-cls_diff, dtype=f32))
    bb = partial[:, P:].reshape(B, 3, P) + bbox_b[P:].reshape(3, P)
    roi_img = np.asarray(roi, dtype=f32) * f32(STRIDE)
    x = roi_img[:, :, 0] - bb[:, 0, :] * roi_img[:, :, 3]
    y = roi_img[:, :, 1]
    w = roi_img[:, :, 2] * np.exp(np.clip(bb[:, 1, :], -10.0, 10.0), dtype=f32)
    hh = roi_img[:, :, 3] * np.exp(np.clip(bb[:, 2, :], -10.0, 10.0),
                                   dtype=f32)
    return np.stack([x, y, w, hh, obj], axis=-1).astype(f32)


def kernel(features, roi, conv_w, conv_b, cls_w, cls_b, bbox_w, bbox_b):
    in_maps, kw = _prep_inputs(features, conv_w, conv_b, cls_w, bbox_w)
    res = _run_device(in_maps)
    partial = np.zeros((B, WCOL), dtype=np.float64)
    for r in res.results:
        partial += np.asarray(r["out"], dtype=np.float64)
    partial *= 2.0 ** (-HSCALE_LOG2 - kw)
    return _postprocess(partial.astype(np.float32), roi, cls_b, bbox_b)


# revision 12
# speedup vs baseline: 2.4156x; 1.0017x over previous
"""Trainium2 Bass kernel for nn_ClassifierModel (nms_detection).

Reference computation:
    h    = relu(features @ conv_w + conv_b)        # (B,H,W,C)@(C,D) -> (B,H,W,D)
    flat = h.reshape(B, F)                         # F = H*W*D = 401408
    cls  = flat @ cls_w + cls_b                    # (B, 64)
    bbox = flat @ bbox_w + bbox_b                  # (B, 128)
    <tiny postprocessing with roi -> (B, P, 5)>

Weight folding (host, exact f32 algebra a compiler could do statically):
  - objectness only needs sigmoid(cls1-cls0), so ship wdiff = cls_w[:,P:] -
    cls_w[:,:P] (32 cols) instead of 64 cls columns.
  - bb[:,:,0] is dead in the reference output (the first x assignment is
    overwritten), so bbox_w[:, :P] is never loaded.
  Device weight matrix = [wdiff | bbox_w[:,P:]]: 128 cols per f-tile.

Precision: all matmul operands are fp8 e4m3 (power-of-2 scaled so values sit
in the normal range; scales undone exactly on the host). PSUM accumulates
f32. Measured end-to-end rel err vs the f32 reference: ~9.6e-3 (gate 2e-2).

Sharding: the flatten (contraction) dim F is split across the 8 cores by
slicing H into 8 chunks of 28 rows (196 pixels/core). Each core computes its
conv slice and a partial (16, 128) product against its slice of the folded
weights; the host sums the 8 partials and runs the tiny postprocessing. Each
dense-weight byte is read exactly once machine-wide, in fp8: ~8.2 MB/core,
which at the 360 GB/s DMA roofline is ~22.7 us — the kernel is HBM-bound.

Per-core layout (fp8 = float8e4):
    xt4  [128,4,3136]: features slice, xt4[p,t,pix*16+b] = feat[c=t*128+p]
    cw4  [128,4,256] : 64*conv_w, cw4[p,t,d]
    cb2  [128,2] f32 : 64*conv_b halves
    wmat [128,392,128]: folded weights * 2^kw, middle index = pix*2 + i
                        (i = d half), cols = [wdiff(32) | bbox1,2,3(96)]
    out  [16,128] f32: partial [cls_diff | bbox123] * 2^(6+kw)

Stage 1 uses fp8 DoubleRow matmuls (two 128-deep c-tiles per matmul) to
produce hQ[p, pix*2+q, b] = fp8(64*h[d=q*128+p, pix, b]) via a strided
activation write. Stage 2 contracts one full pixel (256 d-values) per
DoubleRow matmul: acc[16,128] += hQ[:,2pix:2pix+2,:].T @ wc[:,2t:2t+2,:].

Schedule details (trace-driven):
  - DMA issue order: xt4 first (its long transfer hides the 625+650 ns
    HWDGE/DGE pipeline of the small cw4/cb2 DMAs), then the W stream.
  - The W stream tapers to a tiny 2-pixel final chunk so only ~54 ns of
    matmul work remains after the last byte (+900 ns DMA-sem latency).
  - 16 DMAs total so the final out DMA lands on the last DMAHW lane the
    exit flush polls (the flush waits lanes in order; this keeps the two
    50 ns lane waits ahead of the long one).
  - The USE_TRIGGER path (SWDGE scatter-add prep + trigger_dma, which would
    skip the ~1.3 us HWDGE+DGE issue latency of the final store) is kept for
    documentation but disabled: the prep ticks a DMASW proc lane that no
    instruction increments, and the SP exit drain waits on it -> deadlock.
"""

import numpy as np

B = 16
H, W, C = 224, 7, 512
D = 256
P = 32
NCORES = 8
HSH = H // NCORES          # 28 rows of H per core
PIX = HSH * W              # 196 pixels per core per batch
FLOC = PIX * D             # 50176 contraction elements per core
NB = PIX * B               # 3136 stage-1 moving columns
NTILE = 448                # stage-1 moving tile (3136 = 7*448)
NTI = NB // NTILE          # 7 stage-1 n-tiles
WCOL = 128                 # device weight columns: 32 clsdiff + 96 bbox
# W-stream chunks in pixels (sum = 196). The tapered tail keeps post-stream
# matmul work tiny while keeping the total DMA count low: HWDGE DMAs
# round-robin over 8 DMAHW lanes and each waits for the DMA 8 positions
# earlier to fully complete (+900 ns sem), so many small DMAs serialize.
CHUNKS = [28] * 6 + [12, 8, 4, 2, 2]
STRIDE = 16.0
HSCALE_LOG2 = 6            # h is carried as 64*h in fp8
USE_TRIGGER = False        # SWDGE scatter-add + trigger_dma for the output:
                           # structurally deadlocks under TileContext (the
                           # prep ticks a DMASW proc lane no instruction
                           # increments, and the SP exit drain waits on it),
                           # so the plain dma_start tail is used instead.

_STATE = {}


def _build_module(reps=1):
    import concourse.mybir as mybir
    import concourse.tile as tile
    from concourse import bacc

    f32 = mybir.dt.float32
    i16 = mybir.dt.int16
    fp8 = mybir.dt.float8e4
    DR = mybir.MatmulPerfMode.DoubleRow
    nc = bacc.Bacc("TRN2", target_bir_lowering=False, debug=False)

    xt4_d = nc.dram_tensor("xt4", [128, 4, NB], fp8, kind="ExternalInput")
    cw4_d = nc.dram_tensor("cw4", [128, 4, D], fp8, kind="ExternalInput")
    cb2_d = nc.dram_tensor("cb2", [128, 2], f32, kind="ExternalInput")
    wmat = nc.dram_tensor("wmat", [128, 2 * PIX, WCOL], fp8,
                          kind="ExternalInput")
    if reps == 1:
        out = nc.dram_tensor("out", [16, WCOL], f32, kind="ExternalOutput")
    else:
        out = nc.dram_tensor("out", [reps, 16, WCOL], f32,
                             kind="ExternalOutput")

    with tile.TileContext(nc) as tc:
        with (
            tc.tile_pool(name="res", bufs=2 if reps > 1 else 1) as res,
            tc.tile_pool(name="winb", bufs=6) as winb,
            tc.tile_pool(name="wins", bufs=5) as wins,
            tc.tile_pool(name="ps1", bufs=7, space="PSUM") as ps1p,
            tc.tile_pool(name="ps2", bufs=1, space="PSUM") as ps2p,
        ):
            for rep in range(reps):
                out_r = out[:] if reps == 1 else out[rep]

                # Output staging: ot spans 128 partitions because the SWDGE
                # scatter-add reads its full 128-partition footprint.
                ot = res.tile([128, 1, WCOL], f32, tag="ot", name="ot")
                dma_sem = None
                if USE_TRIGGER:
                    idxs = res.tile([128, 1], i16, tag="idxs", name="idxs")
                    nc.gpsimd.iota(idxs[:], pattern=[[1, 1]], base=0,
                                   channel_multiplier=1)
                    nc.gpsimd.memset(ot[:], 0.0)

                # xt4 first: its long transfer hides the DGE pipelining of
                # the small DMAs behind it.
                xt4 = res.tile([128, 4, NB], fp8, tag="xt4", name="xt4")
                nc.sync.dma_start(xt4[:], xt4_d[:])
                cw4 = res.tile([128, 4, D], fp8, tag="cw4", name="cw4")
                nc.sync.dma_start(cw4[:], cw4_d[:])
                cb2 = res.tile([128, 2], f32, tag="cb2", name="cb2")
                nc.sync.dma_start(cb2[:], cb2_d[:])
                if USE_TRIGGER:
                    # Zero DRAM out early; the final scatter ADDs into it.
                    # Emitted BEFORE the prep so the write-write ordering is
                    # carried by the trigger (the prep defers its data deps),
                    # not by an unsatisfiable wait on the prep's DMASW lane.
                    nc.sync.dma_start(out_r, ot[0:16, 0, :])
                    dma_sem = nc.alloc_semaphore(f"out_dma_{rep}")
                    nc.gpsimd.dma_scatter_add(
                        out_r, ot[:], idxs[:], 16, 16, WCOL,
                        prepare_only=True, sem=dma_sem,
                    )
                wcs = []
                pos = 0
                for c, npix in enumerate(CHUNKS):
                    pool = winb if npix > 12 else wins
                    wc = pool.tile([128, 2 * npix, WCOL], fp8,
                                   tag=f"wc{npix}", name=f"wc{c}")
                    nc.sync.dma_start(
                        wc[:], wmat[:, 2 * pos:2 * (pos + npix), :])
                    wcs.append(wc)
                    pos += npix

                hQ = res.tile([128, 2 * PIX, 16], fp8, tag="hQ", name="hQ")

                # Stage 1: hQ[:, 2*pix+q, b] = relu(64*conv + 64*b) in fp8.
                # One DoubleRow matmul contracts two 128-deep c-tiles.
                for n in range(NTI):
                    for q in range(2):
                        ps = ps1p.tile([128, NTILE // 16, 16], f32, tag="ps",
                                       name=f"ps{n}_{q}")
                        for j in range(2):
                            nc.tensor.matmul(
                                ps[:],
                                cw4[:, 2 * j:2 * j + 2,
                                    q * 128:(q + 1) * 128],
                                xt4[:, 2 * j:2 * j + 2,
                                    n * NTILE:(n + 1) * NTILE],
                                start=(j == 0),
                                stop=(j == 1),
                                perf_mode=DR,
                            )
                        nc.scalar.activation(
                            hQ[:, 2 * n * (NTILE // 16) + q:
                               2 * (n + 1) * (NTILE // 16):2, :],
                            ps[:],
                            mybir.ActivationFunctionType.Relu,
                            bias=cb2[:, q:q + 1],
                        )

                # Stage 2: one DoubleRow matmul per pixel contracts all 256
                # d-values: acc += hQ-pair.T @ W-pair.
                acc = ps2p.tile([16, WCOL], f32, tag="acc", name="acc")
                pix = 0
                for c, npix in enumerate(CHUNKS):
                    for t in range(npix):
                        nc.tensor.matmul(
                            acc[:],
                            hQ[:, 2 * pix:2 * pix + 2, :],
                            wcs[c][:, 2 * t:2 * t + 2, :],
                            start=(pix == 0),
                            stop=(pix == PIX - 1),
                            perf_mode=DR,
                        )
                        pix += 1

                nc.vector.tensor_copy(ot[0:16, 0, :], acc[:])
                if USE_TRIGGER:
                    nc.gpsimd.trigger_dma(count=None)
                    nc.gpsimd.wait_ge(dma_sem, 16 * (rep + 1))
                else:
                    nc.sync.dma_start(out_r, ot[0:16, 0, :])

    nc.compile()
    return nc


def _prep_inputs(features, conv_w, conv_b, cls_w, bbox_w):
    import ml_dtypes

    f32 = np.float32
    fp8 = ml_dtypes.float8_e4m3

    conv_w = np.asarray(conv_w, dtype=f32)
    conv_b = np.asarray(conv_b, dtype=f32)
    cls_w = np.asarray(cls_w, dtype=f32)
    bbox_w = np.asarray(bbox_w, dtype=f32)
    features = np.asarray(features, dtype=f32)

    # Folded, power-of-2-scaled weights (scale undone exactly on the host).
    wfull = np.concatenate([cls_w[:, P:] - cls_w[:, :P], bbox_w[:, P:]],
                           axis=1)  # (F, 128)
    kw = int(np.floor(np.log2(224.0 / np.abs(wfull).max())))
    wq = (wfull * f32(2.0 ** kw)).astype(fp8)

    cw4 = (conv_w * f32(64.0)).astype(fp8).reshape(4, 128, D)
    cw4 = np.ascontiguousarray(cw4.transpose(1, 0, 2))          # (128,4,D)
    cb2 = np.ascontiguousarray((conv_b * f32(64.0)).reshape(2, 128).T)

    in_maps = []
    for i in range(NCORES):
        fi = features[:, i * HSH:(i + 1) * HSH, :, :].astype(fp8)
        fi = fi.reshape(B, PIX, C).transpose(2, 1, 0).reshape(4, 128, NB)
        xt4 = np.ascontiguousarray(fi.transpose(1, 0, 2))       # (128,4,NB)

        wl = wq[i * FLOC:(i + 1) * FLOC].reshape(PIX, 2, 128, WCOL)
        wl = np.ascontiguousarray(wl.transpose(2, 0, 1, 3))     # (128,PIX,2,W)

        in_maps.append({
            "xt4": xt4,
            "cw4": cw4,
            "cb2": cb2,
            "wmat": wl.reshape(128, 2 * PIX, WCOL),
        })
    return in_maps, kw


def _run_device(in_maps, trace=False, **kw):
    from concourse.bass_utils import run_bass_kernel_spmd

    if "nc" not in _STATE:
        _STATE["nc"] = _build_module()
    nc = _STATE["nc"]
    return run_bass_kernel_spmd(
        nc, in_maps, core_ids=list(range(NCORES)), trace=trace, **kw
    )


def _postprocess(partial, roi, cls_b, bbox_b):
    f32 = np.float32
    cls_b = np.asarray(cls_b, dtype=f32)
    bbox_b = np.asarray(bbox_b, dtype=f32)

    cls_diff = partial[:, :P] + (cls_b[P:] - cls_b[:P])
    obj = 1.0 / (1.0 + np.exp(-cls_diff, dtype=f32))
    bb = partial[:, P:].reshape(B, 3, P) + bbox_b[P:].reshape(3, P)
    roi_img = np.asarray(roi, dtype=f32) * f32(STRIDE)
    x = roi_img[:, :, 0] - bb[:, 0, :] * roi_img[:, :, 3]
    y = roi_img[:, :, 1]
    w = roi_img[:, :, 2] * np.exp(np.clip(bb[:, 1, :], -10.0, 10.0), dtype=f32)
    hh = roi_img[:, :, 3] * np.exp(np.clip(bb[:, 2, :], -10.0, 10.0),
                                   dtype=f32)
    return np.stack([x, y, w, hh, obj], axis=-1).astype(f32)


def kernel(features, roi, conv_w, conv_b, cls_w, cls_b, bbox_w, bbox_b):
    in_maps, kw = _prep_inputs(features, conv_w, conv_b, cls_w, bbox_w)
    res = _run_device(in_maps)
    partial = np.zeros((B, WCOL), dtype=np.float64)
    for r in res.results:
        partial += np.asarray(r["out"], dtype=np.float64)
    partial *= 2.0 ** (-HSCALE_LOG2 - kw)
    return _postprocess(partial.astype(np.float32), roi, cls_b, bbox_b)
